# revision 1
# baseline (speedup 1.0000x reference)
"""Trainium2 Bass kernel for the DiscreteAgent GNN (NNConv + LN + MLP head).

Strategy (8 NeuronCores, SPMD, no collectives):
  * Edges are bucketed by destination node range: core c owns dst nodes
    [c*6250, (c+1)*6250) and receives exactly the edges pointing into that
    range.  Each core therefore produces its own disjoint slice of the
    output -> no all-reduce is required at all.
  * Within a core, edges are sorted by destination block (128 nodes per
    block) and padded to a fixed per-block capacity C.  segment_sum becomes,
    per node block, a chain of one-hot matmuls accumulated in PSUM
    (onehot[e, n_local]^T @ msg[e, :]), fully deterministic.
  * Host-side work is index/layout manipulation only (sort, pad, transpose,
    gather of x rows by edge_src); every FLOP runs on the device.

Dtype choices: the w_pre matmul and the one-hot scatter run in fp16 (PE
single-pass + fast weight load; one-hot 0/1 and x are exact/rounded at
~5e-4); the root matmul runs in float32r; LayerNorm and all accumulations
stay in f32 (PSUM is always f32).  Measured end-to-end relative error vs
the fp32 reference: ~9e-4.

The loop is software-pipelined: the scatter stage trails the
w_pre/relu/multiply front by EDGE_LAG tiles, and the node phase is a
4-deep block pipeline (LN -> transpose -> 3 MLP matmuls), so the in-order
tensor engine never waits on freshly produced operands.

Per-core device pipeline, per 128-edge tile (DMAs batched 8 tiles/group):
  PE:  w_pre = [edge_attr|1]^T @ [We;be]  (K=9, f32r) -> PSUM [128, 512]
  ACT: w_relu = relu(w_pre)               -> SBUF
  DVE/GPSIMD: prod = w_relu * broadcast(x_src)   ([128, 32, 16], i inner, fp16)
  DVE: onehot = (iota_row == dst_local)   -> [128, 128] (fp16 out)
  PE:  agg_exp_psum += onehot^T @ prod    (N=512 K-chain over block's tiles)
Per 128-node block:
  PE:  root = [x|1]^T @ [Wroot;bconv] (f32r); DVE: h = reduce_i(agg_exp)+root
  DVE/ACT: LayerNorm + relu
  PE:  featT = Wlin^T @ hreluT ; q1T = Wq1^T @ featT ; qT = Wq2^T @ q1rT
       (fp16, transposed layout; biases folded into aug rows / ACT bias)
  out: qT slice [32, 128] -> DRAM
"""

import sys

import numpy as np

# concourse (Bass/Tile) ships with the container image; make sure it resolves
# even if PYTHONPATH was not inherited.
try:
    import concourse  # noqa: F401
except ImportError:  # pragma: no cover
    for _p in ("/opt/trn_rl_repo", "/opt/pypackages"):
        if _p not in sys.path:
            sys.path.insert(0, _p)

# ---- problem constants (hardcoded per contract) ----
N = 50000
E = 200000
IN_C = 16
HID_C = 32
EDGE_D = 8
OUT_C = 32
MLP_H = 128
N_ACT = 32

M = 8                 # cores
P = 128               # partitions
NPC = N // M          # 6250 nodes per core
NB = (NPC + P - 1) // P   # 49 blocks per core
NPC_PAD = NB * P      # 6272
G = 8                 # edge tiles per DMA group

_PROGRAM_CACHE: dict = {}


def _build_program(C: int, gpsimd_frac: int):
    """Build + compile the SPMD Bass program for per-block edge capacity C.

    gpsimd_frac: out of 4 edge tiles, how many run their broadcast-multiply
    on the GPSIMD engine instead of DVE (load balancing).
    """
    import concourse.tile as tile
    from concourse import bacc, mybir
    from concourse.masks import make_identity

    f32 = mybir.dt.float32
    f32r = mybir.dt.float32r
    fp16 = mybir.dt.float16
    i32 = mybir.dt.int32
    KT = C // P             # K-tiles (edge tiles) per node block
    ET = NB * KT            # edge tiles per core
    EPC = NB * C            # padded edge slots per core

    nc = bacc.Bacc("TRN2", target_bir_lowering=False, debug=False, num_devices=M)

    # --- DRAM I/O (per core) ---
    attrT = nc.dram_tensor("attrT", [EDGE_D + 1, EPC], fp16, kind="ExternalInput")
    xjg = nc.dram_tensor("xjg", [EPC, IN_C], fp16, kind="ExternalInput")
    dstl = nc.dram_tensor("dstl", [EPC], f32, kind="ExternalInput")
    xsT = nc.dram_tensor("xsT", [IN_C + 1, NPC_PAD], fp16, kind="ExternalInput")
    weA = nc.dram_tensor("weA", [EDGE_D + 1, IN_C * HID_C], fp16, kind="ExternalInput")
    wrootA = nc.dram_tensor("wrootA", [IN_C + 1, HID_C], fp16, kind="ExternalInput")
    wlin = nc.dram_tensor("wlin", [HID_C, OUT_C], fp16, kind="ExternalInput")
    wq1 = nc.dram_tensor("wq1", [OUT_C, MLP_H], fp16, kind="ExternalInput")
    wq2 = nc.dram_tensor("wq2", [MLP_H, N_ACT], fp16, kind="ExternalInput")
    bq1c = nc.dram_tensor("bq1c", [MLP_H, 1], f32, kind="ExternalInput")
    bq2c = nc.dram_tensor("bq2c", [N_ACT, 1], f32, kind="ExternalInput")
    gammab = nc.dram_tensor("gammab", [P, HID_C], f32, kind="ExternalInput")
    betab = nc.dram_tensor("betab", [P, HID_C], f32, kind="ExternalInput")
    qT = nc.dram_tensor("qT", [N_ACT, NPC_PAD], f32, kind="ExternalOutput")

    with tile.TileContext(nc) as tc:
        with (
            tc.tile_pool(name="const", bufs=1) as cpool,
            tc.tile_pool(name="edge_in", bufs=3) as epool,
            tc.tile_pool(name="wrelu", bufs=8) as wpool,
            tc.tile_pool(name="work", bufs=8) as kpool,
            tc.tile_pool(name="node", bufs=3) as npool,
            tc.tile_pool(name="wpre_ps", bufs=2, space="PSUM") as wpre_ps,
            tc.tile_pool(name="agg_ps", bufs=2, space="PSUM") as agg_ps,
            tc.tile_pool(name="node_ps", bufs=4, space="PSUM") as node_ps,
        ):
            group_state = {}      # g -> (attr_g, xj_g, dstl_g)
            NGROUPS = (ET + G - 1) // G

            def emit_group_load(g):
                gs = min(G, ET - g * G)
                esl = slice(g * G * P, (g * G + gs) * P)
                attr_g = epool.tile([EDGE_D + 1, G * P], fp16, tag="attr")
                nc.sync.dma_start(attr_g[:, :gs * P], attrT.ap()[:, esl])
                xj_g = epool.tile([P, G, IN_C], fp16, tag="xj")
                nc.sync.dma_start(
                    xj_g[:, :gs, :],
                    xjg.ap()[esl, :].rearrange("(tt p) i -> p tt i", p=P))
                dstl_g = epool.tile([P, G], f32, tag="dstl")
                nc.sync.dma_start(
                    dstl_g[:, :gs],
                    dstl.ap()[esl, None].rearrange("(tt p) o -> p (tt o)", p=P))
                group_state[g] = (attr_g, xj_g, dstl_g)

            # prime the first edge-DMA groups ahead of the big const loads
            for g in range(2):
                emit_group_load(g)

            # ---- persistent constants in SBUF ----
            we_sb = cpool.tile([EDGE_D + 1, IN_C * HID_C], fp16, tag="we")
            nc.sync.dma_start(we_sb[:], weA.ap()[:])
            xsT_sb = cpool.tile([IN_C + 1, NPC_PAD], fp16, tag="xsT")
            nc.sync.dma_start(xsT_sb[:], xsT.ap()[:])
            wroot_sb = cpool.tile([IN_C + 1, HID_C], fp16, tag="wroot")
            nc.sync.dma_start(wroot_sb[:], wrootA.ap()[:])
            wlin_sb = cpool.tile([HID_C, OUT_C], fp16, tag="wlin")
            nc.sync.dma_start(wlin_sb[:], wlin.ap()[:])
            wq1_sb = cpool.tile([OUT_C, MLP_H], fp16, tag="wq1")
            nc.sync.dma_start(wq1_sb[:], wq1.ap()[:])
            wq2_sb = cpool.tile([MLP_H, N_ACT], fp16, tag="wq2")
            nc.sync.dma_start(wq2_sb[:], wq2.ap()[:])
            bq1_sb = cpool.tile([MLP_H, 1], f32, tag="bq1")
            nc.sync.dma_start(bq1_sb[:], bq1c.ap()[:])
            bq2_sb = cpool.tile([N_ACT, 1], f32, tag="bq2")
            nc.sync.dma_start(bq2_sb[:], bq2c.ap()[:])
            gamma_sb = cpool.tile([P, HID_C], f32, tag="gamma")
            nc.sync.dma_start(gamma_sb[:], gammab.ap()[:])
            beta_sb = cpool.tile([P, HID_C], f32, tag="beta")
            nc.sync.dma_start(beta_sb[:], betab.ap()[:])

            # iota row constant: every partition holds [0, 1, ..., 127]
            iota_i = cpool.tile([P, P], i32, tag="iota_i")
            nc.gpsimd.iota(iota_i[:], pattern=[[1, P]], base=0, channel_multiplier=0)
            iota_f = cpool.tile([P, P], fp16, tag="iota_f")
            nc.vector.tensor_copy(iota_f[:], iota_i[:])
            # identity for PE transpose
            ident = cpool.tile([P, P], f32, tag="ident")
            make_identity(nc, ident[:])
            # layernorm epsilon as a per-partition scalar const
            eps_c = cpool.tile([P, 1], f32, tag="eps")
            nc.gpsimd.memset(eps_c[:], 1e-5)

            # ---- software-pipelined stages ----
            EDGE_LAG = 4          # scatter trails prod by this many tiles
            edge_state = {}       # t -> (prod, onehot)
            agg_by_block = {}     # b -> agg psum tile
            blk_state = {}        # b -> dict of node-phase tiles

            def emit_front(t):
                g, tt = divmod(t, G)
                if tt == 0 and g not in group_state:
                    emit_group_load(g)
                if tt == 0 and g + 1 < NGROUPS and g + 1 not in group_state:
                    emit_group_load(g + 1)
                attr_g, xj_g, dstl_g = group_state[g]

                wpre = wpre_ps.tile([P, IN_C * HID_C], f32, tag="wpre")
                nc.tensor.matmul(wpre[:], lhsT=attr_g[:, tt * P:(tt + 1) * P],
                                 rhs=we_sb[:], start=True, stop=True)
                wrelu = wpool.tile([P, IN_C * HID_C], fp16, tag="wrelu")
                nc.scalar.activation(wrelu[:], wpre[:],
                                     mybir.ActivationFunctionType.Relu)
                prod = wpool.tile([P, IN_C * HID_C], fp16, tag="prod")
                xj_b = xj_g[:, tt, :].unsqueeze(1).to_broadcast([P, HID_C, IN_C])
                prod_3d = prod[:].rearrange("p (h i) -> p h i", h=HID_C)
                wrelu_3d = wrelu[:].rearrange("p (h i) -> p h i", h=HID_C)
                if t % 4 < gpsimd_frac:
                    nc.gpsimd.tensor_tensor(prod_3d, wrelu_3d, xj_b,
                                            op=mybir.AluOpType.mult)
                else:
                    nc.vector.tensor_tensor(prod_3d, wrelu_3d, xj_b,
                                            op=mybir.AluOpType.mult)
                onehot = kpool.tile([P, P], fp16, tag="onehot")
                nc.vector.tensor_scalar(
                    onehot[:], iota_f[:], dstl_g[:, tt:tt + 1], None,
                    op0=mybir.AluOpType.is_equal)
                edge_state[t] = (prod, onehot)

            def emit_scatter(t):
                b, kt = divmod(t, KT)
                prod, onehot = edge_state.pop(t)
                if kt == 0:
                    agg_by_block[b] = agg_ps.tile([P, IN_C * HID_C], f32,
                                                  tag="agg", name="agg")
                nc.tensor.matmul(agg_by_block[b][:], lhsT=onehot[:],
                                 rhs=prod[:],
                                 start=(kt == 0), stop=(kt == KT - 1))

            def emit_n1(b):
                # root matmul + i-reduction + LayerNorm -> hrelu
                st = {}
                nsl = slice(b * P, (b + 1) * P)
                agg_cur = agg_by_block.pop(b)
                r_full = node_ps.tile([MLP_H, P], f32, tag="nps")
                root_ps = r_full[:, :HID_C]
                nc.tensor.matmul(root_ps[:], lhsT=xsT_sb[:, nsl],
                                 rhs=wroot_sb[:], start=True, stop=True)
                h0 = npool.tile([P, HID_C], f32, tag="h0")
                nc.vector.tensor_reduce(
                    h0[:], agg_cur[:].rearrange("p (h i) -> p h i", h=HID_C),
                    axis=mybir.AxisListType.X, op=mybir.AluOpType.add)
                hfull = npool.tile([P, HID_C], f32, tag="hfull")
                nc.vector.tensor_add(hfull[:], h0[:], root_ps[:])
                musum = npool.tile([P, 1], f32, tag="musum")
                nc.vector.tensor_reduce(musum[:], hfull[:],
                                        axis=mybir.AxisListType.X,
                                        op=mybir.AluOpType.add)
                negmu = npool.tile([P, 1], f32, tag="negmu")
                nc.scalar.mul(negmu[:], musum[:], -1.0 / HID_C)
                hc = npool.tile([P, HID_C], f32, tag="hc")
                nc.vector.tensor_scalar(hc[:], hfull[:], negmu[:, :1], None,
                                        op0=mybir.AluOpType.add)
                sq = npool.tile([P, HID_C], f32, tag="sq")
                varsum = npool.tile([P, 1], f32, tag="varsum")
                nc.scalar.activation(sq[:], hc[:],
                                     mybir.ActivationFunctionType.Square,
                                     accum_out=varsum[:])
                std = npool.tile([P, 1], f32, tag="std")
                nc.scalar.activation(std[:], varsum[:],
                                     mybir.ActivationFunctionType.Sqrt,
                                     scale=1.0 / HID_C, bias=eps_c[:, :1])
                rstd = npool.tile([P, 1], f32, tag="rstd")
                nc.vector.reciprocal(rstd[:], std[:])
                t1 = npool.tile([P, HID_C], f32, tag="t1")
                nc.vector.tensor_scalar(t1[:], hc[:], rstd[:, :1], None,
                                        op0=mybir.AluOpType.mult)
                t2 = npool.tile([P, HID_C], f32, tag="t2")
                nc.vector.tensor_mul(t2[:], t1[:], gamma_sb[:])
                t3 = npool.tile([P, HID_C], f32, tag="t3")
                nc.vector.tensor_add(t3[:], t2[:], beta_sb[:])
                hrelu = npool.tile([P, HID_C], f32, tag="hrelu")
                nc.scalar.activation(hrelu[:], t3[:],
                                     mybir.ActivationFunctionType.Relu)
                st["hrelu"] = hrelu
                blk_state[b] = st

            def emit_n2a(b):
                st = blk_state[b]
                hT_full = node_ps.tile([MLP_H, P], f32, tag="nps")
                hT_ps = hT_full[:HID_C]
                nc.tensor.transpose(hT_ps[:], st.pop("hrelu")[:], ident[:])
                hT = npool.tile([HID_C, P], fp16, tag="hTs")
                nc.scalar.copy(hT[:], hT_ps[:])
                st["hT"] = hT

            def emit_n2b(b):
                st = blk_state[b]
                fT_full = node_ps.tile([MLP_H, P], f32, tag="nps")
                fT_ps = fT_full[:OUT_C]
                nc.tensor.matmul(fT_ps[:], lhsT=wlin_sb[:], rhs=st.pop("hT")[:],
                                 start=True, stop=True)
                fT = npool.tile([OUT_C, P], fp16, tag="fTs")
                nc.scalar.copy(fT[:], fT_ps[:])
                st["fT"] = fT

            def emit_n2c(b):
                st = blk_state[b]
                q1_ps = node_ps.tile([MLP_H, P], f32, tag="nps")
                nc.tensor.matmul(q1_ps[:], lhsT=wq1_sb[:], rhs=st.pop("fT")[:],
                                 start=True, stop=True)
                q1r = npool.tile([MLP_H, P], fp16, tag="q1r")
                nc.scalar.activation(q1r[:], q1_ps[:],
                                     mybir.ActivationFunctionType.Relu,
                                     bias=bq1_sb[:, :1])
                st["q1r"] = q1r

            def emit_n2d(b):
                st = blk_state.pop(b)
                q_full = node_ps.tile([MLP_H, P], f32, tag="nps")
                q_ps = q_full[:N_ACT]
                nc.tensor.matmul(q_ps[:], lhsT=wq2_sb[:], rhs=st.pop("q1r")[:],
                                 start=True, stop=True)
                qfin = npool.tile([N_ACT, P], f32, tag="qfin")
                nc.vector.tensor_scalar(qfin[:], q_ps[:], bq2_sb[:, :1], None,
                                        op0=mybir.AluOpType.add)
                nsl = slice(b * P, (b + 1) * P)
                nc.sync.dma_start(qT.ap()[:, nsl], qfin[:])

            for s in range(ET + EDGE_LAG):
                if s < ET:
                    emit_front(s)
                t = s - EDGE_LAG
                if t >= 0:
                    emit_scatter(t)
                    if t % KT == KT - 1:
                        b = t // KT
                        emit_n1(b)
                        if b >= 1:
                            emit_n2a(b - 1)
                        if b >= 2:
                            emit_n2b(b - 2)
                        if b >= 3:
                            emit_n2c(b - 3)
                        if b >= 4:
                            emit_n2d(b - 4)
            # drain the block pipeline
            emit_n2a(NB - 1)
            emit_n2b(NB - 2)
            emit_n2c(NB - 3)
            emit_n2d(NB - 4)
            emit_n2b(NB - 1)
            emit_n2c(NB - 2)
            emit_n2d(NB - 3)
            emit_n2c(NB - 1)
            emit_n2d(NB - 2)
            emit_n2d(NB - 1)

    nc.compile()
    return nc


def _get_program(C: int, gpsimd_frac: int):
    key = (C, gpsimd_frac)
    if key not in _PROGRAM_CACHE:
        _PROGRAM_CACHE[key] = _build_program(C, gpsimd_frac)
    return _PROGRAM_CACHE[key]


def _prep_inputs(x, edge_src, edge_dst, edge_attr,
                 We, be, Wroot, bconv, gamma, beta,
                 Wlin, blin, Wq1, bq1, Wq2, bq2):
    """Host-side sharding: bucket+sort edges by destination, pad per block,
    build per-core input maps. Index/layout work only."""
    f32 = np.float32
    x = np.asarray(x, f32)
    edge_src = np.asarray(edge_src)
    edge_dst = np.asarray(edge_dst)
    edge_attr = np.asarray(edge_attr, f32)

    order = np.argsort(edge_dst, kind="stable")
    dst_s = edge_dst[order]
    src_s = edge_src[order]
    attr_s = edge_attr[order]

    core_of = dst_s // NPC
    local = dst_s - core_of * NPC
    blk = local // P
    gblk = core_of * NB + blk
    counts = np.bincount(gblk, minlength=M * NB)
    C = int(max(256, -(-counts.max() // P) * P))
    EPC = NB * C

    starts = np.zeros(M * NB, np.int64)
    starts[1:] = np.cumsum(counts)[:-1]
    rank = np.arange(E, dtype=np.int64) - starts[gblk]
    slot = gblk.astype(np.int64) * C + rank    # into [M*NB*C]

    tot = M * NB * C
    attr_all = np.zeros((tot, EDGE_D + 1), np.float16)
    attr_all[slot, :EDGE_D] = attr_s
    attr_all[slot, EDGE_D] = 1.0
    xj_all = np.zeros((tot, IN_C), np.float16)
    xj_all[slot] = x[src_s].astype(np.float16)
    dstl_all = np.full(tot, -1.0, f32)
    dstl_all[slot] = (local % P).astype(f32)

    attr_all = attr_all.reshape(M, EPC, EDGE_D + 1)
    xj_all = xj_all.reshape(M, EPC, IN_C)
    dstl_all = dstl_all.reshape(M, EPC)

    # node-slice features, augmented with ones row, transposed
    x_pad = np.zeros((M, NPC_PAD, IN_C + 1), np.float16)
    x_resh = x.reshape(M, NPC, IN_C)
    x_pad[:, :NPC, :IN_C] = x_resh
    x_pad[:, :, IN_C] = 1.0

    # parameters (replicated)
    We = np.asarray(We, f32)
    be = np.asarray(be, f32)
    Wroot = np.asarray(Wroot, f32)
    bconv = np.asarray(bconv, f32)
    gamma = np.asarray(gamma, f32)
    beta = np.asarray(beta, f32)
    Wlin = np.asarray(Wlin, f32)
    blin = np.asarray(blin, f32)
    Wq1 = np.asarray(Wq1, f32)
    bq1 = np.asarray(bq1, f32)
    Wq2 = np.asarray(Wq2, f32)
    bq2 = np.asarray(bq2, f32)

    weA = np.concatenate([We, be[None, :]], axis=0)            # [9, 512]
    # permute columns from (i, h) to (h, i) layout so the device-side
    # i-reduction is over the contiguous innermost dim
    weA_perm = np.ascontiguousarray(
        weA.reshape(EDGE_D + 1, IN_C, HID_C).transpose(0, 2, 1)
           .reshape(EDGE_D + 1, IN_C * HID_C)).astype(np.float16)
    wrootA = np.concatenate([Wroot, bconv[None, :]], axis=0).astype(np.float16)
    bq1p = (blin @ Wq1 + bq1).astype(f32)                      # blin folded
    gammab = np.broadcast_to(gamma, (P, HID_C)).copy()
    betab = np.broadcast_to(beta, (P, HID_C)).copy()

    in_maps = []
    for c in range(M):
        in_maps.append({
            "attrT": np.ascontiguousarray(attr_all[c].T),
            "xjg": np.ascontiguousarray(xj_all[c]),
            "dstl": np.ascontiguousarray(dstl_all[c]),
            "xsT": np.ascontiguousarray(x_pad[c].T),
            "weA": weA_perm,
            "wrootA": wrootA,
            "wlin": Wlin.astype(np.float16),
            "wq1": Wq1.astype(np.float16),
            "wq2": Wq2.astype(np.float16),
            "bq1c": bq1p[:, None],
            "bq2c": bq2[:, None],
            "gammab": gammab,
            "betab": betab,
        })
    return C, in_maps


GPSIMD_FRAC = 1  # of every 4 edge tiles, how many multiply on GPSIMD


def kernel(**inputs) -> np.ndarray:
    from concourse.bass_utils import run_bass_kernel_spmd

    C, in_maps = _prep_inputs(**inputs)
    nc = _get_program(C, GPSIMD_FRAC)
    res = run_bass_kernel_spmd(nc, in_maps, list(range(M)))
    q = np.empty((N, N_ACT), np.float32)
    for c in range(M):
        q[c * NPC:(c + 1) * NPC] = res.results[c]["qT"][:, :NPC].T
    return q



# revision 16
# speedup vs baseline: 1.3739x; 1.3739x over previous
"""Trainium2 Bass kernel for the DiscreteAgent GNN (NNConv + LN + MLP head).

Strategy (8 NeuronCores, SPMD, no collectives):
  * Nodes are partitioned by range across cores (core c owns [c*6250,
    (c+1)*6250)); edges follow their destination node, so each core
    produces a disjoint slice of the output and no all-reduce is needed.
  * Within a core, nodes are re-packed into NB=50 blocks of <=128 nodes
    via degree-sorted serpentine dealing, so every block receives <=512
    edges.  Each block therefore owns exactly KT=4 fixed 128-edge tiles:
    a single static SPMD program covers all cores with ~2% edge padding.
  * Per tile, the per-edge message msg[e,h] = sum_i x_src[e,i]*w_relu[e,i,h]
    is formed by a broadcast multiply (DVE) and an i-reduction (mostly
    GPSIMD, which is otherwise idle); segment_sum is then a cheap N=32
    one-hot matmul accumulated in PSUM per block.  The one-hot matrices
    are precomputed host-side and DMA'd (index/layout work only).
  * gnn.lin is folded into q_head's first layer host-side (parameter-only
    algebra); LayerNorm and the MLP head run batched over groups of 4
    node blocks (one 128x128 transpose + 512-column matmuls per group).

The tensor engine runs at a sustained ~1.2GHz (~1.04ns/column); the
kernel keeps its per-tile work to one 512-column matmul (the edge-MLP)
plus a 32-column scatter, with relu on ACT and multiply/reduce split
across DVE/GPSIMD to balance all four engines.

Per-core device pipeline, per 128-edge tile (DMAs batched 8 tiles/group):
  PE:  w_pre = attr_aug^T @ [We;be]  (K=9, fp16) -> PSUM [128, 512]
  ACT: w_relu = relu(w_pre)          -> SBUF fp16  [128, (i,h)]
  DVE: prod = w_relu * broadcast(x_src)
  DVE/GPSIMD: msg = reduce_i(prod)   -> SBUF fp16 [128, 32]
  PE:  agg_psum += onehot^T @ msg    (N=32, K-chain over the block's tiles)
Per 128-node block:
  PE: root = x_aug^T @ [Wroot;bconv]; DVE: h = agg + root
Per 4-block group (512 nodes):
  DVE/ACT: LayerNorm + relu (batched); PE: one [128,128] transpose
  PE: q1T = Wc4^T @ hreluT (4 PE-tiled matmuls); ACT relu
  PE: qT = Wq2^T @ q1rT; ACT copy; DMA out
"""

import sys

import numpy as np

try:
    import concourse  # noqa: F401
except ImportError:  # pragma: no cover
    for _p in ("/opt/trn_rl_repo", "/opt/pypackages"):
        if _p not in sys.path:
            sys.path.insert(0, _p)

# ---- problem constants (hardcoded per contract) ----
N = 50000
E = 200000
IN_C = 16
HID_C = 32
EDGE_D = 8
OUT_C = 32
MLP_H = 128
N_ACT = 32

M = 8                 # cores
P = 128               # partitions
NPC = N // M          # 6250 nodes per core
KT = 4                # edge tiles per node block (block edge cap = 512)
G = 8                 # edge tiles per DMA group
EDGE_LAG = 6          # scatter trails the front by this many tiles

# per-tile engine assignment (by t % 8): which slots run on GPSIMD
GP_MULT_SLOTS = ()
GP_FOLD_SLOTS = (0, 1, 2, 3, 4, 5, 6, 7)   # 16->8 fold on GPSIMD

_PROGRAM_CACHE: dict = {}


def _build_program(NB: int, flags: tuple):
    """Build + compile the SPMD Bass program.

    NB: node blocks per core; flags: (has_gb, has_bc, has_bq2).
    """
    import concourse.tile as tile
    from concourse import bacc, mybir
    from concourse.masks import make_identity

    has_gb, has_bc, has_bq2 = flags
    f32 = mybir.dt.float32
    fp16 = mybir.dt.float16
    ET = NB * KT
    EPC = ET * P
    NPC_PAD = NB * P
    NGROUPS = (ET + G - 1) // G

    nc = bacc.Bacc("TRN2", target_bir_lowering=False, debug=False, num_devices=M)

    # --- DRAM I/O (per core) ---
    attrT = nc.dram_tensor("attrT", [EDGE_D + 1, EPC], fp16, kind="ExternalInput")
    xjp = nc.dram_tensor("xjp", [P, ET * IN_C], fp16, kind="ExternalInput")
    ohp = nc.dram_tensor("ohp", [P, ET * P], fp16, kind="ExternalInput")
    xsT = nc.dram_tensor("xsT", [IN_C + 1, NPC_PAD], fp16, kind="ExternalInput")
    weA = nc.dram_tensor("weA", [EDGE_D + 1, IN_C * HID_C], fp16, kind="ExternalInput")
    wrootA = nc.dram_tensor("wrootA", [IN_C + 1, HID_C], fp16, kind="ExternalInput")
    wc4 = nc.dram_tensor("wc4", [P, MLP_H], fp16, kind="ExternalInput")
    wq2 = nc.dram_tensor("wq2", [MLP_H, N_ACT], fp16, kind="ExternalInput")
    gb = nc.dram_tensor("gb", [P, 4 * HID_C], f32, kind="ExternalInput")
    bb = nc.dram_tensor("bb", [P, 4 * HID_C], f32, kind="ExternalInput")
    bcc = nc.dram_tensor("bcc", [MLP_H, 1], f32, kind="ExternalInput")
    bq2c = nc.dram_tensor("bq2c", [N_ACT, 1], f32, kind="ExternalInput")
    qT = nc.dram_tensor("qT", [N_ACT, NPC_PAD], f32, kind="ExternalOutput")

    with tile.TileContext(nc) as tc:
        with (
            tc.tile_pool(name="const", bufs=1) as cpool,
            tc.tile_pool(name="edge_in", bufs=3) as epool,
            tc.tile_pool(name="wrelu", bufs=4) as wpool,
            tc.tile_pool(name="prod", bufs=4) as ppool,
            tc.tile_pool(name="prodh", bufs=4) as phpool,
            tc.tile_pool(name="msg", bufs=8) as mpool,
            tc.tile_pool(name="node", bufs=2) as npool,
            tc.tile_pool(name="wpre_ps", bufs=2, space="PSUM") as wpre_ps,
            tc.tile_pool(name="agg_ps", bufs=2, space="PSUM") as agg_ps,
            tc.tile_pool(name="q1_ps", bufs=1, space="PSUM") as q1_ps,
            tc.tile_pool(name="sm_ps", bufs=1, space="PSUM") as sm_ps,
            tc.tile_pool(name="tr_ps", bufs=1, space="PSUM") as tr_ps,
            tc.tile_pool(name="q2_ps", bufs=1, space="PSUM") as q2_ps,
        ):
            group_state = {}      # g -> (attr_g, xj_g, oh_g)

            def emit_group_load(g):
                gs = min(G, ET - g * G)
                esl = slice(g * G * P, (g * G + gs) * P)
                attr_g = epool.tile([EDGE_D + 1, G * P], fp16, tag="attr")
                nc.sync.dma_start(attr_g[:, :gs * P], attrT.ap()[:, esl])
                xj_g = epool.tile([P, G, IN_C], fp16, tag="xj")
                nc.sync.dma_start(
                    xj_g[:, :gs, :],
                    xjp.ap()[:, g * G * IN_C:(g * G + gs) * IN_C]
                       .rearrange("p (tt i) -> p tt i", i=IN_C))
                oh_g = epool.tile([P, G, P], fp16, tag="oh")
                nc.sync.dma_start(
                    oh_g[:, :gs, :],
                    ohp.ap()[:, g * G * P:(g * G + gs) * P]
                       .rearrange("p (tt n) -> p tt n", n=P))
                group_state[g] = (attr_g, xj_g, oh_g)

            # prime the first edge-DMA groups ahead of the big const loads
            for g in range(2):
                emit_group_load(g)

            # ---- persistent constants in SBUF ----
            we_sb = cpool.tile([EDGE_D + 1, IN_C * HID_C], fp16, tag="we")
            nc.sync.dma_start(we_sb[:], weA.ap()[:])
            xsT_sb = cpool.tile([IN_C + 1, NPC_PAD], fp16, tag="xsT")
            nc.sync.dma_start(xsT_sb[:], xsT.ap()[:])
            wroot_sb = cpool.tile([IN_C + 1, HID_C], fp16, tag="wroot")
            nc.sync.dma_start(wroot_sb[:], wrootA.ap()[:])
            wc4_sb = cpool.tile([P, MLP_H], fp16, tag="wc4")
            nc.sync.dma_start(wc4_sb[:], wc4.ap()[:])
            wq2_sb = cpool.tile([MLP_H, N_ACT], fp16, tag="wq2")
            nc.sync.dma_start(wq2_sb[:], wq2.ap()[:])
            if has_gb:
                gb_sb = cpool.tile([P, 4 * HID_C], f32, tag="gb")
                nc.sync.dma_start(gb_sb[:], gb.ap()[:])
                bb_sb = cpool.tile([P, 4 * HID_C], f32, tag="bb")
                nc.sync.dma_start(bb_sb[:], bb.ap()[:])
            if has_bc:
                bc_sb = cpool.tile([MLP_H, 1], f32, tag="bc")
                nc.sync.dma_start(bc_sb[:], bcc.ap()[:])
            if has_bq2:
                bq2_sb = cpool.tile([N_ACT, 1], f32, tag="bq2")
                nc.sync.dma_start(bq2_sb[:], bq2c.ap()[:])

            ident = cpool.tile([P, P], f32, tag="ident")
            make_identity(nc, ident[:])
            eps_c = cpool.tile([P, 1], f32, tag="eps")
            nc.gpsimd.memset(eps_c[:], 1e-5)

            msg_state = {}        # t -> msg tile
            agg_by_block = {}     # b -> agg psum tile
            grp_state = {}        # g -> dict of group node-phase tiles
            tasks = {}            # emission step -> [fn]

            def at(step, fn):
                tasks.setdefault(step, []).append(fn)

            def emit_front(s):
                g, tt = divmod(s, G)
                if g not in group_state:
                    emit_group_load(g)
                if tt == 0 and g + 1 < NGROUPS and g + 1 not in group_state:
                    emit_group_load(g + 1)
                attr_g, xj_g, oh_g = group_state[g]

                wpre = wpre_ps.tile([P, IN_C * HID_C], f32, tag="wpre")
                nc.tensor.matmul(wpre[:], lhsT=attr_g[:, tt * P:(tt + 1) * P],
                                 rhs=we_sb[:], start=True, stop=True)
                wrelu = wpool.tile([P, IN_C * HID_C], fp16, tag="wrelu")
                nc.scalar.activation(wrelu[:], wpre[:],
                                     mybir.ActivationFunctionType.Relu)
                # layout is [p, (i, h)] with h innermost? No: [p, (h, i)] with
                # i innermost (weA columns are permuted host-side) so the
                # i-reduction is over the contiguous inner dim.
                prod = ppool.tile([P, IN_C * HID_C], fp16, tag="prod")
                xj_b = xj_g[:, tt, :].unsqueeze(1).to_broadcast(
                    [P, HID_C, IN_C])
                prod_3d = prod[:].rearrange("p (h i) -> p h i", h=HID_C)
                wrelu_3d = wrelu[:].rearrange("p (h i) -> p h i", h=HID_C)
                meng = nc.gpsimd if (s % 8) in GP_MULT_SLOTS else nc.vector
                meng.tensor_tensor(prod_3d, wrelu_3d, xj_b,
                                   op=mybir.AluOpType.mult)
                # 16 -> 8 binary fold (GPSIMD), then 8 -> 1 reduce (DVE)
                prodh = phpool.tile([P, HID_C, IN_C // 2], fp16, tag="prodh")
                feng = nc.gpsimd if (s % 8) in GP_FOLD_SLOTS else nc.vector
                with nc.allow_low_precision(reason="fp16 msg (16-term sums)"):
                    feng.tensor_tensor(prodh[:], prod_3d[:, :, :IN_C // 2],
                                       prod_3d[:, :, IN_C // 2:],
                                       op=mybir.AluOpType.add)
                    msg = mpool.tile([P, HID_C], fp16, tag="msg")
                    nc.vector.tensor_reduce(msg[:], prodh[:],
                                            axis=mybir.AxisListType.X,
                                            op=mybir.AluOpType.add)
                msg_state[s] = msg

            def emit_scatter(t):
                b, kt = divmod(t, KT)
                g, tt = divmod(t, G)
                oh_g = group_state[g][2]
                msg = msg_state.pop(t)
                if kt == 0:
                    agg_by_block[b] = agg_ps.tile([P, HID_C], f32, tag="agg",
                                                  name="agg")
                nc.tensor.matmul(agg_by_block[b][:], lhsT=oh_g[:, tt, :],
                                 rhs=msg[:],
                                 start=(kt == 0), stop=False)
                if kt == KT - 1:
                    # fold the root matmul into the same PSUM accumulation
                    nsl = slice(b * P, (b + 1) * P)
                    nc.tensor.matmul(agg_by_block[b][:],
                                     lhsT=xsT_sb[:, nsl], rhs=wroot_sb[:],
                                     start=False, stop=True)

            def emit_block_tail(b):
                g, j = divmod(b, 4)
                if j == 0:
                    st = grp_state[g] = {}
                    st["h0"] = npool.tile([P, 4, HID_C], f32,
                                          tag="h0", name="h0")
                st = grp_state[g]
                agg = agg_by_block.pop(b)
                nc.scalar.copy(st["h0"][:, j, :], agg[:])

            def emit_ln(g):
                st = grp_state[g]
                gsz = min(4, NB - g * 4)
                hfull = st.pop("h0")
                musum = npool.tile([P, 4], f32, tag="musum")
                nc.vector.tensor_reduce(musum[:, :gsz], hfull[:, :gsz, :],
                                        axis=mybir.AxisListType.X,
                                        op=mybir.AluOpType.add)
                negmu = npool.tile([P, 4], f32, tag="negmu")
                nc.scalar.mul(negmu[:, :gsz], musum[:, :gsz], -1.0 / HID_C)
                hc = npool.tile([P, 4, HID_C], f32, tag="hc")
                nc.vector.tensor_tensor(
                    hc[:, :gsz, :], hfull[:, :gsz, :],
                    negmu[:, :gsz].unsqueeze(2).to_broadcast([P, gsz, HID_C]),
                    op=mybir.AluOpType.add)
                sq = npool.tile([P, 4, HID_C], f32, tag="sq")
                nc.scalar.activation(sq[:, :gsz, :], hc[:, :gsz, :],
                                     mybir.ActivationFunctionType.Square)
                varsum = npool.tile([P, 4], f32, tag="varsum")
                nc.vector.tensor_reduce(varsum[:, :gsz], sq[:, :gsz, :],
                                        axis=mybir.AxisListType.X,
                                        op=mybir.AluOpType.add)
                std = npool.tile([P, 4], f32, tag="std")
                nc.scalar.activation(std[:, :gsz], varsum[:, :gsz],
                                     mybir.ActivationFunctionType.Sqrt,
                                     scale=1.0 / HID_C, bias=eps_c[:, :1])
                rstd = npool.tile([P, 4], f32, tag="rstd")
                nc.vector.reciprocal(rstd[:, :gsz], std[:, :gsz])
                t1 = npool.tile([P, 4, HID_C], f32, tag="t1")
                nc.vector.tensor_tensor(
                    t1[:, :gsz, :], hc[:, :gsz, :],
                    rstd[:, :gsz].unsqueeze(2).to_broadcast([P, gsz, HID_C]),
                    op=mybir.AluOpType.mult)
                if has_gb:
                    t2 = npool.tile([P, 4, HID_C], f32, tag="t2")
                    nc.vector.tensor_tensor(
                        t2[:, :gsz, :], t1[:, :gsz, :],
                        gb_sb[:].rearrange("p (j h) -> p j h", j=4)[:, :gsz, :],
                        op=mybir.AluOpType.mult)
                    t3 = npool.tile([P, 4, HID_C], f32, tag="t3")
                    nc.vector.tensor_tensor(
                        t3[:, :gsz, :], t2[:, :gsz, :],
                        bb_sb[:].rearrange("p (j h) -> p j h", j=4)[:, :gsz, :],
                        op=mybir.AluOpType.add)
                    t1 = t3
                hrelu = npool.tile([P, 4, HID_C], f32, tag="hrelu")
                nc.scalar.activation(hrelu[:, :gsz, :], t1[:, :gsz, :],
                                     mybir.ActivationFunctionType.Relu)
                st["hrelu"] = hrelu

            def emit_transp(g):
                st = grp_state[g]
                gsz = min(4, NB - g * 4)
                trp = tr_ps.tile([HID_C, 4 * P], f32, tag="trp")
                hrelu = st.pop("hrelu")
                for j in range(gsz):
                    nc.tensor.transpose(trp[:, j * P:(j + 1) * P],
                                        hrelu[:, j, :], ident[:])
                hT = npool.tile([HID_C, 4 * P], fp16, tag="hT")
                nc.vector.tensor_copy(hT[:, :gsz * P], trp[:, :gsz * P])
                st["hT"] = hT
                st["gsz"] = gsz

            def emit_q1(g):
                st = grp_state[g]
                gsz = st["gsz"]
                q1p = q1_ps.tile([MLP_H, 4 * P], f32, tag="q1p")
                nc.tensor.matmul(q1p[:, :gsz * P],
                                 lhsT=wc4_sb[:HID_C, :],
                                 rhs=st.pop("hT")[:, :gsz * P],
                                 start=True, stop=True)
                st["q1p"] = q1p

            def emit_q1r(g):
                st = grp_state[g]
                gsz = st["gsz"]
                q1r = npool.tile([MLP_H, 4 * P], fp16, tag="q1r")
                if has_bc:
                    nc.scalar.activation(q1r[:, :gsz * P],
                                         st.pop("q1p")[:, :gsz * P],
                                         mybir.ActivationFunctionType.Relu,
                                         bias=bc_sb[:, :1])
                else:
                    nc.scalar.activation(q1r[:, :gsz * P],
                                         st.pop("q1p")[:, :gsz * P],
                                         mybir.ActivationFunctionType.Relu)
                st["q1r"] = q1r

            def emit_q2_out(g):
                st = grp_state.pop(g)
                gsz = st["gsz"]
                q2p = q2_ps.tile([N_ACT, 4 * P], f32, tag="q2p")
                nc.tensor.matmul(q2p[:, :gsz * P], lhsT=wq2_sb[:],
                                 rhs=st.pop("q1r")[:, :gsz * P],
                                 start=True, stop=True)
                qf = npool.tile([N_ACT, 4 * P], f32, tag="qf")
                if has_bq2:
                    nc.scalar.activation(qf[:, :gsz * P], q2p[:, :gsz * P],
                                         mybir.ActivationFunctionType.Identity,
                                         bias=bq2_sb[:, :1])
                else:
                    nc.scalar.copy(qf[:, :gsz * P], q2p[:, :gsz * P])
                osl = slice(g * 4 * P, g * 4 * P + gsz * P)
                nc.sync.dma_start(qT.ap()[:, osl], qf[:, :gsz * P])

            # ---- software-pipelined emission ----
            TOTAL = ET + EDGE_LAG + 8
            for s in range(TOTAL):
                if s < ET:
                    emit_front(s)
                t = s - EDGE_LAG
                if 0 <= t < ET:
                    emit_scatter(t)
                    b, kt = divmod(t, KT)
                    if kt == KT - 1:
                        emit_block_tail(b)
                        if b % 4 == 3 or b == NB - 1:
                            g = b // 4
                            at(s + 1, lambda g=g: emit_ln(g))
                            at(s + 2, lambda g=g: emit_transp(g))
                            at(s + 3, lambda g=g: emit_q1(g))
                            at(s + 4, lambda g=g: emit_q1r(g))
                            at(s + 5, lambda g=g: emit_q2_out(g))
                for fn in tasks.pop(s, []):
                    fn()
            assert not tasks and not grp_state and not agg_by_block

    nc.compile()
    return nc


def _get_program(NB: int, flags: tuple):
    key = (NB, flags, GP_MULT_SLOTS, GP_FOLD_SLOTS, EDGE_LAG, G)
    if key not in _PROGRAM_CACHE:
        _PROGRAM_CACHE[key] = _build_program(NB, flags)
    return _PROGRAM_CACHE[key]


def _pack_nodes(deg_core: np.ndarray, NB: int):
    """Serpentine-deal nodes (desc degree) into NB blocks of <=128 nodes.
    Returns (blk, col) per local node, or None if a block exceeds KT*P
    edges."""
    npc = deg_core.shape[0]
    order = np.argsort(-deg_core, kind="stable")
    k = np.arange(npc)
    rnd, pos = k // NB, k % NB
    bin_ = np.where(rnd % 2 == 0, pos, NB - 1 - pos)
    col = rnd
    loads = np.bincount(bin_, weights=deg_core[order], minlength=NB)
    nodes = np.bincount(bin_, minlength=NB)
    if loads.max() > KT * P or nodes.max() > P:
        return None
    blk = np.empty(npc, np.int64)
    colv = np.empty(npc, np.int64)
    blk[order] = bin_
    colv[order] = col
    return blk, colv


def _prep_inputs(x, edge_src, edge_dst, edge_attr,
                 We, be, Wroot, bconv, gamma, beta,
                 Wlin, blin, Wq1, bq1, Wq2, bq2):
    """Host-side sharding: node re-packing, edge bucketing/sorting/padding,
    one-hot construction, parameter folding. Index/layout work only (the
    only arithmetic is on parameters: Wlin@Wq1 folding)."""
    f32 = np.float32
    x = np.asarray(x, f32)
    edge_src = np.asarray(edge_src)
    edge_dst = np.asarray(edge_dst)
    edge_attr = np.asarray(edge_attr, f32)

    deg = np.bincount(edge_dst, minlength=N)

    NB = 50
    packs = []
    while True:
        ok = True
        packs = []
        for c in range(M):
            r = _pack_nodes(deg[c * NPC:(c + 1) * NPC], NB)
            if r is None:
                ok = False
                break
            packs.append(r)
        if ok:
            break
        NB += 1
        assert NB <= 64, "node packing failed"

    ET = NB * KT
    EPC = ET * P
    NPC_PAD = NB * P
    C = KT * P

    # node placement per core
    nodeid_pad = np.full((M, NPC_PAD), -1, np.int64)
    for c in range(M):
        blk, colv = packs[c]
        nodeid_pad[c, blk * P + colv] = np.arange(NPC) + c * NPC

    # per-edge (core, block, col)
    e_core = edge_dst // NPC
    dst_local = edge_dst - e_core * NPC

    attr_all = np.zeros((M, EPC, EDGE_D + 1), np.float16)
    xj_all = np.zeros((M, EPC, IN_C), np.float16)
    oh_all = np.zeros((M, EPC, P), np.float16)
    x16 = x.astype(np.float16)

    for c in range(M):
        blk, colv = packs[c]
        sel = np.nonzero(e_core == c)[0]
        eb = blk[dst_local[sel]]
        ecol = colv[dst_local[sel]]
        order = np.argsort(eb, kind="stable")
        sel = sel[order]
        eb = eb[order]
        ecol = ecol[order]
        cnt = np.bincount(eb, minlength=NB)
        starts = np.zeros(NB, np.int64)
        starts[1:] = np.cumsum(cnt)[:-1]
        rank = np.arange(sel.shape[0]) - starts[eb]
        slot = eb * C + rank
        attr_all[c, slot, :EDGE_D] = edge_attr[sel]
        attr_all[c, slot, EDGE_D] = 1.0
        xj_all[c, slot] = x16[edge_src[sel]]
        oh_all[c, slot, ecol] = 1.0

    # p-major DMA layouts
    attrT_m = np.ascontiguousarray(attr_all.transpose(0, 2, 1))
    xjp_m = np.ascontiguousarray(
        xj_all.reshape(M, ET, P, IN_C).transpose(0, 2, 1, 3)
    ).reshape(M, P, ET * IN_C)
    ohp_m = np.ascontiguousarray(
        oh_all.reshape(M, ET, P, P).transpose(0, 2, 1, 3)
    ).reshape(M, P, ET * P)

    # node features per core (packed order), augmented with ones row
    x_pad = np.zeros((M, NPC_PAD, IN_C + 1), np.float16)
    for c in range(M):
        valid = nodeid_pad[c] >= 0
        x_pad[c, valid, :IN_C] = x16[nodeid_pad[c, valid]]
    x_pad[:, :, IN_C] = 1.0
    xsT_m = np.ascontiguousarray(x_pad.transpose(0, 2, 1))

    # parameters (replicated); layout/fold work on host
    We = np.asarray(We, f32)
    be = np.asarray(be, f32)
    Wroot = np.asarray(Wroot, f32)
    bconv = np.asarray(bconv, f32)
    gamma = np.asarray(gamma, f32)
    beta = np.asarray(beta, f32)
    Wlin = np.asarray(Wlin, f32)
    blin = np.asarray(blin, f32)
    Wq1 = np.asarray(Wq1, f32)
    bq1 = np.asarray(bq1, f32)
    Wq2 = np.asarray(Wq2, f32)
    bq2 = np.asarray(bq2, f32)

    weA = np.concatenate([We, be[None, :]], axis=0)            # [9, 512]
    # permute columns from (i, h) to (h, i) so the device-side i-reduction
    # is over the contiguous innermost dim
    weA_perm = np.ascontiguousarray(
        weA.reshape(EDGE_D + 1, IN_C, HID_C).transpose(0, 2, 1)
           .reshape(EDGE_D + 1, IN_C * HID_C)).astype(np.float16)
    wrootA = np.concatenate([Wroot, bconv[None, :]], axis=0).astype(np.float16)
    wc = (Wlin @ Wq1).astype(np.float16)                       # [32, 128]
    wc4 = np.ascontiguousarray(np.tile(wc, (4, 1)))            # [128, 128]
    bc = (blin @ Wq1 + bq1).astype(f32)

    has_gb = not (np.all(gamma == 1.0) and np.all(beta == 0.0))
    has_bc = bool(np.any(bc != 0.0))
    has_bq2 = bool(np.any(bq2 != 0.0))
    flags = (has_gb, has_bc, has_bq2)

    gb_m = np.ascontiguousarray(
        np.broadcast_to(np.tile(gamma, 4), (P, 4 * HID_C)).astype(f32))
    bb_m = np.ascontiguousarray(
        np.broadcast_to(np.tile(beta, 4), (P, 4 * HID_C)).astype(f32))

    in_maps = []
    for c in range(M):
        in_maps.append({
            "attrT": attrT_m[c],
            "xjp": xjp_m[c],
            "ohp": ohp_m[c],
            "xsT": xsT_m[c],
            "weA": weA_perm,
            "wrootA": wrootA,
            "wc4": wc4,
            "wq2": Wq2.astype(np.float16),
            "gb": gb_m,
            "bb": bb_m,
            "bcc": bc[:, None],
            "bq2c": bq2[:, None].astype(f32),
        })
    return NB, flags, nodeid_pad, in_maps


def kernel(**inputs) -> np.ndarray:
    from concourse.bass_utils import run_bass_kernel_spmd

    NB, flags, nodeid_pad, in_maps = _prep_inputs(**inputs)
    nc = _get_program(NB, flags)
    res = run_bass_kernel_spmd(nc, in_maps, list(range(M)))
    q = np.empty((N, N_ACT), np.float32)
    for c in range(M):
        valid = nodeid_pad[c] >= 0
        q[nodeid_pad[c, valid]] = res.results[c]["qT"][:, valid].T
    return q
